# revision 12
# baseline (speedup 1.0000x reference)
"""Dcls1d via overlap-save rFFT conv on 8 Trainium2 NeuronCores.

F=256 overlap-save FFT convolution, data-parallel over batch (4/core):
  fwd:  per segment (21/batch, V=201 valid outs), DFT as 2-chain matmuls
        -> x_hat[c, bins] in SBUF (bf16). xs loads grouped 3 segs/DMA,
        alternating the two HW-DGE queues (sync/scalar).
  pw:   per bin k, complex pointwise mult-accumulate over c as matmuls;
        D_hat streamed from DRAM in 4-bin groups with 8KB-contiguous
        descriptors, alternating both HW queues. -Xi negations batched
        8 bins at a time on vector. psum drains: Re->scalar, Im->vector.
        Corner-turn flushes (psum-partition s -> oh-partition bin) as
        per-row SBUF->SBUF DMAs split between gpsimd and sync.
  inv:  per segment, irFFT matmuls + bias, staged 3 segs then one bf16
        DMA out per group, alternating queues (y written bf16; host
        upcasts -- well inside the 2e-2 tolerance).

Host precomputes D_hat = conj(rfft(D_dense, 256)) in bf16.
"""

import numpy as np
import ml_dtypes
from contextlib import ExitStack

import concourse.bacc as bacc
import concourse.mybir as mybir
import concourse.tile as tile
from concourse.bass_utils import run_bass_kernel_spmd

DT = mybir.dt
BF = ml_dtypes.bfloat16

B, CIN, COUT, L = 32, 256, 256, 4096
KTAPS, DIL, PAD = 7, 8, 28
LD = KTAPS * DIL  # 56
TOUT = L + 1  # 4097
NCORES = 8
BPC = B // NCORES  # 4

F = 256
NBIN = F // 2 + 1  # 129
V = F - LD + 1  # 201
SEGS = (TOUT + V - 1) // V  # 21
YL = SEGS * V  # 4221
LPAD = V * (SEGS - 1) + F  # 4276

XG = 3   # segs per xs load
DG = 4   # bins per dht load
NG = 8   # bins per negation batch
YG = 3   # segs per y store

_nc_cache = {}


def build_dense_kernel(weight: np.ndarray, P: np.ndarray) -> np.ndarray:
    """Scatter taps into dense [O, C, LD] kernel (fp32-exact vs reference)."""
    w = weight.astype(np.float32)
    pos = np.clip(P.astype(np.float32) + np.float32(LD // 2), np.float32(0.0), np.float32(LD - 1))
    lo = np.floor(pos)
    frac = pos - lo
    lo_i = lo.astype(np.int64)
    hi_i = np.minimum(lo_i + 1, LD - 1)
    O, C, K = w.shape
    oi = np.arange(O)[:, None, None]
    ci = np.arange(C)[None, :, None]
    D = np.zeros((O, C, LD), np.float32)
    np.add.at(D, (oi, ci, lo_i), w * (np.float32(1.0) - frac))
    np.add.at(D, (oi, ci, hi_i), w * frac)
    return D


def build_consts(D):
    """Wf [256,256], WI [256,V], Dh [129,2,C,O] (fp32; cast at use)."""
    t = np.arange(F)[:, None]
    k = np.arange(NBIN)[None, :]
    ang = 2 * np.pi * t * k / F
    Wf = np.concatenate([np.cos(ang), -np.sin(ang[:, 1:128])], axis=1)
    m = np.arange(V)[None, :]
    kk = np.arange(NBIN)[:, None]
    alpha = np.where((kk == 0) | (kk == NBIN - 1), 1.0, 2.0) / F
    angi = 2 * np.pi * kk * m / F
    WI = np.concatenate([alpha * np.cos(angi), -(alpha * np.sin(angi))[1:128]], axis=0)
    Kh = np.conj(np.fft.rfft(D, n=F, axis=2))  # [O,C,129]
    Dh = np.stack([Kh.real, Kh.imag], axis=0)  # [2,O,C,129]
    Dh = np.ascontiguousarray(np.transpose(Dh, (3, 0, 2, 1)))  # [129,2,C,O]
    return Wf.astype(np.float32), WI.astype(np.float32), Dh.astype(np.float32)


def build_nc(bpc=BPC):
    S = bpc * SEGS
    nc = bacc.Bacc("TRN2", target_bir_lowering=False, debug=False)
    # xt[b, tp, seg, tcn, c] -- per (b,tp) all segs contiguous
    xt_d = nc.dram_tensor("xt", [bpc, 128, SEGS, 2, CIN], DT.bfloat16, kind="ExternalInput").ap()
    # dh[cp, k, cb, ri, o] -- per partition all bins contiguous; bin-pair
    # loads give 4KB descriptors, pairs alternate the two HW-DGE queues
    dh_d = nc.dram_tensor("dh", [128, NBIN, 2, 2, COUT], DT.bfloat16, kind="ExternalInput").ap()
    wf_d = nc.dram_tensor("wf", [2, 128, 256], DT.bfloat16, kind="ExternalInput").ap()
    wi_d = nc.dram_tensor("wi", [2, 128, V], DT.bfloat16, kind="ExternalInput").ap()
    bias_d = nc.dram_tensor("bias", [128, 2], DT.float32, kind="ExternalInput").ap()
    # y[b, op, ot, t] bf16 -- host upcasts and reorders
    y_d = nc.dram_tensor("y", [bpc, 128, 2, YL], DT.bfloat16, kind="ExternalOutput").ap()

    with ExitStack() as ctx:
        tc = ctx.enter_context(tile.TileContext(nc))
        cpool = ctx.enter_context(tc.tile_pool(name="c", bufs=1))

        wft = cpool.tile([128, 2, 256], DT.bfloat16)
        wit = cpool.tile([128, 2, V], DT.bfloat16)
        biast = cpool.tile([128, 2], DT.float32)
        for tcn in range(2):
            nc.scalar.dma_start(wft[:, tcn, :], wf_d[tcn])
            nc.scalar.dma_start(wit[:, tcn, :], wi_d[tcn])
        nc.scalar.dma_start(biast[:], bias_d[:])

        # persistent SBUF stores
        xh = cpool.tile([128, 2, 256, S], DT.bfloat16, name="xh", tag="xh")
        ohA = cpool.tile([128, S, 256], DT.bfloat16, name="ohA", tag="ohA")
        ohB = cpool.tile([128, S, 256], DT.bfloat16, name="ohB", tag="ohB")

        # ---- forward DFT ----
        with tc.tile_pool(name="x", bufs=4) as xpool, \
             tc.tile_pool(name="psF", bufs=4, space="PSUM") as psF:
            qrr = 0
            for b in range(bpc):
                for g in range(SEGS // XG):
                    xs = xpool.tile([128, XG, 2, CIN], DT.bfloat16)
                    eng = nc.sync if qrr % 2 == 0 else nc.scalar
                    qrr += 1
                    eng.dma_start(xs[:], xt_d[b, :, g * XG : (g + 1) * XG])
                    for j in range(XG):
                        s = b * SEGS + g * XG + j
                        pf = psF.tile([128, 2, 256], DT.float32)
                        for cb in range(2):
                            for tcn in range(2):
                                nc.tensor.matmul(
                                    pf[:, cb, :],
                                    xs[:, j, tcn, cb * 128 : (cb + 1) * 128],
                                    wft[:, tcn, :],
                                    start=(tcn == 0),
                                    stop=(tcn == 1),
                                    skip_group_check=True,
                                )
                        if s % 2 == 0:
                            nc.vector.tensor_copy(xh[:, :, :, s], pf[:])
                        else:
                            nc.scalar.copy(xh[:, :, :, s], pf[:])

        # ---- pointwise complex multiply ----
        GRP = 2
        with tc.tile_pool(name="dh", bufs=6) as dhpool, \
             tc.tile_pool(name="ng", bufs=2) as ngpool, \
             tc.tile_pool(name="st", bufs=3) as stpool, \
             tc.tile_pool(name="psPW", bufs=4, space="PSUM") as psPW:
            stA = stB = None
            stA_rows = stB_rows = None

            def flush(st, rows, oh, eng):
                if st is None or not rows:
                    return
                # partition dim must stay outermost in SBUF APs, so scatter
                # row-by-row: dst [1, S, 256] <- src [S, 1, 256]
                for j, r in enumerate(rows):
                    eng.dma_start(oh[r : r + 1], st[:, j : j + 1, :])

            ng = None
            ng_k0 = -NG
            dht = None
            for k in range(NBIN):
                jd = k % 2
                if jd == 0:
                    dht = dhpool.tile([128, 2, 2, 2, COUT], DT.bfloat16)
                    kd = min(2, NBIN - k)
                    eng = nc.sync if (k // 2) % 2 == 0 else nc.scalar
                    eng.dma_start(dht[:, :kd], dh_d[:, k : k + kd])
                has_im = 0 < k < NBIN - 1
                if has_im and k >= ng_k0 + NG:
                    ng_k0 = k
                    ng = ngpool.tile([128, 2, NG, S], DT.bfloat16)
                    kn = min(NG, NBIN - 1 - k)
                    for cb in range(2):
                        nc.vector.tensor_scalar_mul(
                            ng[:, cb, :kn, :], xh[:, cb, 128 + k : 128 + k + kn, :], -1.0
                        )
                ps = psPW.tile([S, 2, 256], DT.float32)
                # Re: Xr*Dr + (-Xi)*Di
                n_acc = 4 if has_im else 2
                idx = 0
                for cb in range(2):
                    nc.tensor.matmul(
                        ps[:, 0, :], xh[:, cb, k, :], dht[:, jd, cb, 0, :],
                        start=(idx == 0), stop=(idx == n_acc - 1),
                        skip_group_check=True,
                    )
                    idx += 1
                if has_im:
                    for cb in range(2):
                        nc.tensor.matmul(
                            ps[:, 0, :], ng[:, cb, k - ng_k0, :], dht[:, jd, cb, 1, :],
                            start=False, stop=(idx == n_acc - 1),
                            skip_group_check=True,
                        )
                        idx += 1
                    # Im: Xr*Di + Xi*Dr
                    idx = 0
                    for cb in range(2):
                        nc.tensor.matmul(
                            ps[:, 1, :], xh[:, cb, k, :], dht[:, jd, cb, 1, :],
                            start=(idx == 0), stop=False,
                            skip_group_check=True,
                        )
                        idx += 1
                    for cb in range(2):
                        nc.tensor.matmul(
                            ps[:, 1, :], xh[:, cb, 128 + k, :], dht[:, jd, cb, 0, :],
                            start=False, stop=(idx == 3),
                            skip_group_check=True,
                        )
                        idx += 1
                # stage Re row (ohA row k, or ohB row 0 for k=128) on scalar
                if k < 128:
                    if stA is None:
                        stA = stpool.tile([S, GRP, 256], DT.bfloat16)
                        stA_rows = []
                    nc.scalar.copy(stA[:, len(stA_rows), :], ps[:, 0, :])
                    stA_rows.append(k)
                    if len(stA_rows) == GRP:
                        flush(stA, stA_rows, ohA, nc.gpsimd)
                        stA = None
                else:
                    stx = stpool.tile([S, 1, 256], DT.bfloat16)
                    nc.scalar.copy(stx[:, 0, :], ps[:, 0, :])
                    flush(stx, [0], ohB, nc.gpsimd)
                # stage Im row (ohB row k) on vector
                if has_im:
                    if stB is None:
                        stB = stpool.tile([S, GRP, 256], DT.bfloat16)
                        stB_rows = []
                    nc.vector.tensor_copy(stB[:, len(stB_rows), :], ps[:, 1, :])
                    stB_rows.append(k)
                    if len(stB_rows) == GRP:
                        flush(stB, stB_rows, ohB, nc.sync)
                        stB = None
            flush(stA, stA_rows, ohA, nc.gpsimd)
            flush(stB, stB_rows, ohB, nc.sync)

        # ---- inverse DFT + bias ----
        with tc.tile_pool(name="y", bufs=3) as ypool, \
             tc.tile_pool(name="psI", bufs=4, space="PSUM") as psI:
            ys = None
            qrr = 0
            for s in range(S):
                b, i = divmod(s, SEGS)
                j = i % YG
                if j == 0:
                    ys = ypool.tile([128, 2, YG, V], DT.bfloat16)
                pv = psI.tile([128, 2, V], DT.float32)
                for ot in range(2):
                    nc.tensor.matmul(
                        pv[:, ot, :], ohA[:, s, ot * 128 : (ot + 1) * 128], wit[:, 0, :],
                        start=True, stop=False, skip_group_check=True,
                    )
                    nc.tensor.matmul(
                        pv[:, ot, :], ohB[:, s, ot * 128 : (ot + 1) * 128], wit[:, 1, :],
                        start=False, stop=True, skip_group_check=True,
                    )
                nc.vector.tensor_scalar_add(
                    ys[:, 0, j, :], pv[:, 0, :], biast[:, 0:1]
                )
                nc.scalar.add(ys[:, 1, j, :], pv[:, 1, :], biast[:, 1:2])
                if j == YG - 1:
                    eng = nc.scalar if qrr % 2 == 0 else nc.sync
                    qrr += 1
                    eng.dma_start(
                        y_d[b, :, :, (i - YG + 1) * V : (i + 1) * V],
                        ys[:],
                    )

    nc.compile()
    return nc


def host_inputs(input, weight, P, bias):
    """Host-side staging: xt segments (transposed, bf16) + spectra consts."""
    D = build_dense_kernel(weight, P)
    Wf, WI, Dh = build_consts(D)
    wf = np.ascontiguousarray(Wf.reshape(2, 128, 256)).astype(BF)
    wi = np.ascontiguousarray(WI.reshape(2, 128, V)).astype(BF)
    # Dh [129, 2ri, C, O] -> dh[cp, k, cb, ri, o]
    dh = np.ascontiguousarray(
        Dh.reshape(NBIN, 2, 2, 128, COUT).transpose(3, 0, 2, 1, 4)
    ).astype(BF)
    bias2 = np.ascontiguousarray(np.asarray(bias, np.float32).reshape(2, 128).T)
    xpad = np.zeros((input.shape[0], CIN, LPAD), np.float32)
    xpad[:, :, PAD : PAD + L] = input
    xpad = xpad.astype(BF)
    idx = V * np.arange(SEGS)[:, None] + np.arange(F)[None, :]
    segs = xpad[:, :, idx]  # [B, C, SEGS, F]
    # [B, SEGS, F, C] -> [B, SEGS, tcn, tp, C] -> [B, tp, SEGS, tcn, C]
    xt = segs.transpose(0, 2, 3, 1).reshape(input.shape[0], SEGS, 2, 128, CIN)
    xt = np.ascontiguousarray(xt.transpose(0, 3, 1, 2, 4))
    return xt, dh, wf, wi, bias2


def make_in_maps(inputs):
    xt, dh, wf, wi, bias2 = host_inputs(
        np.ascontiguousarray(inputs["input"], np.float32),
        inputs["weight"],
        inputs["P"],
        inputs["bias"],
    )
    return [
        {
            "xt": np.ascontiguousarray(xt[i * BPC : (i + 1) * BPC]),
            "dh": dh,
            "wf": wf,
            "wi": wi,
            "bias": bias2,
        }
        for i in range(NCORES)
    ]


def kernel(input, weight, P, bias):
    if "nc" not in _nc_cache:
        _nc_cache["nc"] = build_nc()
    nc = _nc_cache["nc"]
    in_maps = make_in_maps(
        {"input": input, "weight": weight, "P": P, "bias": bias}
    )
    res = run_bass_kernel_spmd(nc, in_maps, core_ids=list(range(NCORES)))
    out = np.concatenate(
        [
            np.asarray(r["y"])
            .astype(np.float32)
            .transpose(0, 2, 1, 3)
            .reshape(BPC, COUT, YL)
            for r in res.results
        ],
        axis=0,
    )
    return np.ascontiguousarray(out[:, :, :TOUT])


# revision 15
# speedup vs baseline: 1.1172x; 1.1172x over previous
"""Dcls1d via overlap-save rFFT conv on 8 Trainium2 NeuronCores.

F=256 overlap-save FFT convolution, data-parallel over batch (4/core):
  fwd:  per segment (21/batch, V=201 valid outs), DFT as 2-chain matmuls
        -> x_hat[c, bins] in SBUF (bf16). xs loads grouped 3 segs/DMA,
        alternating the two HW-DGE queues (sync/scalar).
  pw:   per bin k, complex pointwise mult-accumulate over c as matmuls;
        D_hat streamed from DRAM in 4-bin groups with 8KB-contiguous
        descriptors, alternating both HW queues. -Xi negations batched
        8 bins at a time on vector. psum drains: Re->scalar, Im->vector.
        Corner-turn flushes (psum-partition s -> oh-partition bin) as
        per-row SBUF->SBUF DMAs split between gpsimd and sync.
  inv:  per segment, irFFT matmuls + bias, staged 3 segs then one bf16
        DMA out per group, alternating queues (y written bf16; host
        upcasts -- well inside the 2e-2 tolerance).

Host precomputes D_hat = conj(rfft(D_dense, 256)) in bf16.
"""

import numpy as np
import ml_dtypes
from contextlib import ExitStack

import concourse.bacc as bacc
import concourse.mybir as mybir
import concourse.tile as tile
from concourse.bass_utils import run_bass_kernel_spmd

DT = mybir.dt
BF = ml_dtypes.bfloat16

B, CIN, COUT, L = 32, 256, 256, 4096
KTAPS, DIL, PAD = 7, 8, 28
LD = KTAPS * DIL  # 56
TOUT = L + 1  # 4097
NCORES = 8
BPC = B // NCORES  # 4

F = 256
NBIN = F // 2 + 1  # 129
V = F - LD + 1  # 201
SEGS = (TOUT + V - 1) // V  # 21
YL = SEGS * V  # 4221
LPAD = V * (SEGS - 1) + F  # 4276

XG = 3   # segs per xs load
DG = 4   # bins per dht load
NG = 8   # bins per negation batch
YG = 3   # segs per y store

_nc_cache = {}


def build_dense_kernel(weight: np.ndarray, P: np.ndarray) -> np.ndarray:
    """Scatter taps into dense [O, C, LD] kernel (fp32-exact vs reference)."""
    w = weight.astype(np.float32)
    pos = np.clip(P.astype(np.float32) + np.float32(LD // 2), np.float32(0.0), np.float32(LD - 1))
    lo = np.floor(pos)
    frac = pos - lo
    lo_i = lo.astype(np.int64)
    hi_i = np.minimum(lo_i + 1, LD - 1)
    O, C, K = w.shape
    oi = np.arange(O)[:, None, None]
    ci = np.arange(C)[None, :, None]
    D = np.zeros((O, C, LD), np.float32)
    np.add.at(D, (oi, ci, lo_i), w * (np.float32(1.0) - frac))
    np.add.at(D, (oi, ci, hi_i), w * frac)
    return D


def build_consts(D):
    """Wf [256,256], WI [256,V], Dh [129,2,C,O] (fp32; cast at use)."""
    t = np.arange(F)[:, None]
    k = np.arange(NBIN)[None, :]
    ang = 2 * np.pi * t * k / F
    Wf = np.concatenate([np.cos(ang), -np.sin(ang[:, 1:128])], axis=1)
    m = np.arange(V)[None, :]
    kk = np.arange(NBIN)[:, None]
    alpha = np.where((kk == 0) | (kk == NBIN - 1), 1.0, 2.0) / F
    angi = 2 * np.pi * kk * m / F
    WI = np.concatenate([alpha * np.cos(angi), -(alpha * np.sin(angi))[1:128]], axis=0)
    Kh = np.conj(np.fft.rfft(D, n=F, axis=2))  # [O,C,129]
    Dh = np.stack([Kh.real, Kh.imag], axis=0)  # [2,O,C,129]
    Dh = np.ascontiguousarray(np.transpose(Dh, (3, 0, 2, 1)))  # [129,2,C,O]
    return Wf.astype(np.float32), WI.astype(np.float32), Dh.astype(np.float32)


def build_nc(bpc=BPC):
    S = bpc * SEGS
    nc = bacc.Bacc("TRN2", target_bir_lowering=False, debug=False)
    # xt[b, tp, seg, tcn, c] -- per (b,tp) all segs contiguous
    xt_d = nc.dram_tensor("xt", [bpc, 128, SEGS, 2, CIN], DT.bfloat16, kind="ExternalInput").ap()
    # dh[cp, k, cb, ri, o] -- per partition all bins contiguous; bin-pair
    # loads give 4KB descriptors, pairs alternate the two HW-DGE queues
    dh_d = nc.dram_tensor("dh", [128, NBIN, 2, 2, COUT], DT.bfloat16, kind="ExternalInput").ap()
    wf_d = nc.dram_tensor("wf", [2, 128, 256], DT.bfloat16, kind="ExternalInput").ap()
    wi_d = nc.dram_tensor("wi", [2, 128, V], DT.bfloat16, kind="ExternalInput").ap()
    bias_d = nc.dram_tensor("bias", [128, 2], DT.float32, kind="ExternalInput").ap()
    # y[b, op, ot, t] bf16 -- host upcasts and reorders
    y_d = nc.dram_tensor("y", [bpc, 128, 2, YL], DT.bfloat16, kind="ExternalOutput").ap()

    with ExitStack() as ctx:
        tc = ctx.enter_context(tile.TileContext(nc))
        cpool = ctx.enter_context(tc.tile_pool(name="c", bufs=1))

        wft = cpool.tile([128, 2, 256], DT.bfloat16)
        wit = cpool.tile([128, 2, V], DT.bfloat16)
        biast = cpool.tile([128, 2], DT.float32)
        for tcn in range(2):
            nc.scalar.dma_start(wft[:, tcn, :], wf_d[tcn])
            nc.scalar.dma_start(wit[:, tcn, :], wi_d[tcn])
        nc.scalar.dma_start(biast[:], bias_d[:])

        # persistent SBUF stores
        xh = cpool.tile([128, 2, 256, S], DT.bfloat16, name="xh", tag="xh")
        ohA = cpool.tile([128, S, 256], DT.bfloat16, name="ohA", tag="ohA")
        ohB = cpool.tile([128, S, 256], DT.bfloat16, name="ohB", tag="ohB")

        # ---- forward DFT ----
        with tc.tile_pool(name="x", bufs=4) as xpool, \
             tc.tile_pool(name="psF", bufs=4, space="PSUM") as psF:
            qrr = 0
            for b in range(bpc):
                for g in range(SEGS // XG):
                    xs = xpool.tile([128, XG, 2, CIN], DT.bfloat16)
                    eng = nc.sync if qrr % 2 == 0 else nc.scalar
                    qrr += 1
                    eng.dma_start(xs[:], xt_d[b, :, g * XG : (g + 1) * XG])
                    for j in range(XG):
                        s = b * SEGS + g * XG + j
                        pf = psF.tile([128, 2, 256], DT.float32)
                        for cb in range(2):
                            for tcn in range(2):
                                nc.tensor.matmul(
                                    pf[:, cb, :],
                                    xs[:, j, tcn, cb * 128 : (cb + 1) * 128],
                                    wft[:, tcn, :],
                                    start=(tcn == 0),
                                    stop=(tcn == 1),
                                    skip_group_check=True,
                                )
                        # drain on vector only -- scalar and sync must stay
                        # pure DMA issuers so xs/dht prefetch runs ahead
                        nc.vector.tensor_copy(xh[:, :, :, s], pf[:])

        # ---- pointwise complex multiply ----
        GRP = 2
        with tc.tile_pool(name="dh", bufs=6) as dhpool, \
             tc.tile_pool(name="ng", bufs=2) as ngpool, \
             tc.tile_pool(name="st", bufs=3) as stpool, \
             tc.tile_pool(name="psPW", bufs=4, space="PSUM") as psPW:
            stA = stB = None
            stA_rows = stB_rows = None

            def flush(st, rows, oh, eng):
                if st is None or not rows:
                    return
                # partition dim must stay outermost in SBUF APs, so scatter
                # row-by-row: dst [1, S, 256] <- src [S, 1, 256]
                for j, r in enumerate(rows):
                    eng.dma_start(oh[r : r + 1], st[:, j : j + 1, :])

            ng = None
            ng_k0 = -NG
            dht = None
            for k in range(NBIN):
                jd = k % 2
                if jd == 0:
                    dht = dhpool.tile([128, 2, 2, 2, COUT], DT.bfloat16)
                    kd = min(2, NBIN - k)
                    eng = nc.sync if (k // 2) % 2 == 0 else nc.scalar
                    eng.dma_start(dht[:, :kd], dh_d[:, k : k + kd])
                has_im = 0 < k < NBIN - 1
                if has_im and k >= ng_k0 + NG:
                    ng_k0 = k
                    ng = ngpool.tile([128, 2, NG, S], DT.bfloat16)
                    kn = min(NG, NBIN - 1 - k)
                    for cb in range(2):
                        nc.vector.tensor_scalar_mul(
                            ng[:, cb, :kn, :], xh[:, cb, 128 + k : 128 + k + kn, :], -1.0
                        )
                ps = psPW.tile([S, 2, 256], DT.float32)
                # Re: Xr*Dr + (-Xi)*Di
                n_acc = 4 if has_im else 2
                idx = 0
                for cb in range(2):
                    nc.tensor.matmul(
                        ps[:, 0, :], xh[:, cb, k, :], dht[:, jd, cb, 0, :],
                        start=(idx == 0), stop=(idx == n_acc - 1),
                        skip_group_check=True,
                    )
                    idx += 1
                if has_im:
                    for cb in range(2):
                        nc.tensor.matmul(
                            ps[:, 0, :], ng[:, cb, k - ng_k0, :], dht[:, jd, cb, 1, :],
                            start=False, stop=(idx == n_acc - 1),
                            skip_group_check=True,
                        )
                        idx += 1
                    # Im: Xr*Di + Xi*Dr
                    idx = 0
                    for cb in range(2):
                        nc.tensor.matmul(
                            ps[:, 1, :], xh[:, cb, k, :], dht[:, jd, cb, 1, :],
                            start=(idx == 0), stop=False,
                            skip_group_check=True,
                        )
                        idx += 1
                    for cb in range(2):
                        nc.tensor.matmul(
                            ps[:, 1, :], xh[:, cb, 128 + k, :], dht[:, jd, cb, 0, :],
                            start=False, stop=(idx == 3),
                            skip_group_check=True,
                        )
                        idx += 1
                # stage Re row (ohA row k, or ohB row 0 for k=128) on scalar
                if k < 128:
                    if stA is None:
                        stA = stpool.tile([S, GRP, 256], DT.bfloat16)
                        stA_rows = []
                    nc.vector.tensor_copy(stA[:, len(stA_rows), :], ps[:, 0, :])
                    stA_rows.append(k)
                    if len(stA_rows) == GRP:
                        flush(stA, stA_rows, ohA, nc.gpsimd)
                        stA = None
                else:
                    stx = stpool.tile([S, 1, 256], DT.bfloat16)
                    nc.vector.tensor_copy(stx[:, 0, :], ps[:, 0, :])
                    flush(stx, [0], ohB, nc.gpsimd)
                # stage Im row (ohB row k) on vector
                if has_im:
                    if stB is None:
                        stB = stpool.tile([S, GRP, 256], DT.bfloat16)
                        stB_rows = []
                    nc.vector.tensor_copy(stB[:, len(stB_rows), :], ps[:, 1, :])
                    stB_rows.append(k)
                    if len(stB_rows) == GRP:
                        flush(stB, stB_rows, ohB, nc.gpsimd)
                        stB = None
            flush(stA, stA_rows, ohA, nc.gpsimd)
            flush(stB, stB_rows, ohB, nc.gpsimd)

        # ---- inverse DFT + bias ----
        with tc.tile_pool(name="y", bufs=3) as ypool, \
             tc.tile_pool(name="psI", bufs=4, space="PSUM") as psI:
            ys = None
            qrr = 0
            for s in range(S):
                b, i = divmod(s, SEGS)
                j = i % YG
                if j == 0:
                    ys = ypool.tile([128, 2, YG, V], DT.bfloat16)
                pv = psI.tile([128, 2, V], DT.float32)
                for ot in range(2):
                    nc.tensor.matmul(
                        pv[:, ot, :], ohA[:, s, ot * 128 : (ot + 1) * 128], wit[:, 0, :],
                        start=True, stop=False, skip_group_check=True,
                    )
                    nc.tensor.matmul(
                        pv[:, ot, :], ohB[:, s, ot * 128 : (ot + 1) * 128], wit[:, 1, :],
                        start=False, stop=True, skip_group_check=True,
                    )
                nc.vector.tensor_scalar_add(
                    ys[:, 0, j, :], pv[:, 0, :], biast[:, 0:1]
                )
                nc.scalar.add(ys[:, 1, j, :], pv[:, 1, :], biast[:, 1:2])
                if j == YG - 1:
                    eng = nc.scalar if qrr % 2 == 0 else nc.sync
                    qrr += 1
                    eng.dma_start(
                        y_d[b, :, :, (i - YG + 1) * V : (i + 1) * V],
                        ys[:],
                    )

    nc.compile()
    return nc


def host_inputs(input, weight, P, bias):
    """Host-side staging: xt segments (transposed, bf16) + spectra consts."""
    D = build_dense_kernel(weight, P)
    Wf, WI, Dh = build_consts(D)
    wf = np.ascontiguousarray(Wf.reshape(2, 128, 256)).astype(BF)
    wi = np.ascontiguousarray(WI.reshape(2, 128, V)).astype(BF)
    # Dh [129, 2ri, C, O] -> dh[cp, k, cb, ri, o]
    dh = np.ascontiguousarray(
        Dh.reshape(NBIN, 2, 2, 128, COUT).transpose(3, 0, 2, 1, 4)
    ).astype(BF)
    bias2 = np.ascontiguousarray(np.asarray(bias, np.float32).reshape(2, 128).T)
    xpad = np.zeros((input.shape[0], CIN, LPAD), np.float32)
    xpad[:, :, PAD : PAD + L] = input
    xpad = xpad.astype(BF)
    idx = V * np.arange(SEGS)[:, None] + np.arange(F)[None, :]
    segs = xpad[:, :, idx]  # [B, C, SEGS, F]
    # [B, SEGS, F, C] -> [B, SEGS, tcn, tp, C] -> [B, tp, SEGS, tcn, C]
    xt = segs.transpose(0, 2, 3, 1).reshape(input.shape[0], SEGS, 2, 128, CIN)
    xt = np.ascontiguousarray(xt.transpose(0, 3, 1, 2, 4))
    return xt, dh, wf, wi, bias2


def make_in_maps(inputs):
    xt, dh, wf, wi, bias2 = host_inputs(
        np.ascontiguousarray(inputs["input"], np.float32),
        inputs["weight"],
        inputs["P"],
        inputs["bias"],
    )
    return [
        {
            "xt": np.ascontiguousarray(xt[i * BPC : (i + 1) * BPC]),
            "dh": dh,
            "wf": wf,
            "wi": wi,
            "bias": bias2,
        }
        for i in range(NCORES)
    ]


def kernel(input, weight, P, bias):
    if "nc" not in _nc_cache:
        _nc_cache["nc"] = build_nc()
    nc = _nc_cache["nc"]
    in_maps = make_in_maps(
        {"input": input, "weight": weight, "P": P, "bias": bias}
    )
    res = run_bass_kernel_spmd(nc, in_maps, core_ids=list(range(NCORES)))
    out = np.concatenate(
        [
            np.asarray(r["y"])
            .astype(np.float32)
            .transpose(0, 2, 1, 3)
            .reshape(BPC, COUT, YL)
            for r in res.results
        ],
        axis=0,
    )
    return np.ascontiguousarray(out[:, :, :TOUT])


# revision 16
# speedup vs baseline: 1.2862x; 1.1513x over previous
"""Dcls1d via overlap-save rFFT conv on 8 Trainium2 NeuronCores.

F=256 overlap-save FFT convolution, data-parallel over batch (4/core):
  fwd:  per segment (21/batch, V=201 valid outs), DFT as 2-chain matmuls
        -> x_hat[c, bins] in SBUF (bf16). xs loads grouped 3 segs/DMA,
        alternating the two HW-DGE queues (sync/scalar).
  pw:   per bin k, complex pointwise mult-accumulate over c as matmuls;
        D_hat streamed from DRAM in 4-bin groups with 8KB-contiguous
        descriptors, alternating both HW queues. -Xi negations batched
        8 bins at a time on vector. psum drains: Re->scalar, Im->vector.
        Corner-turn flushes (psum-partition s -> oh-partition bin) as
        per-row SBUF->SBUF DMAs split between gpsimd and sync.
  inv:  per segment, irFFT matmuls + bias, staged 3 segs then one bf16
        DMA out per group, alternating queues (y written bf16; host
        upcasts -- well inside the 2e-2 tolerance).

Host precomputes D_hat = conj(rfft(D_dense, 256)) in bf16.
"""

import numpy as np
import ml_dtypes
from contextlib import ExitStack

import concourse.bacc as bacc
import concourse.mybir as mybir
import concourse.tile as tile
from concourse.bass_utils import run_bass_kernel_spmd

DT = mybir.dt
BF = ml_dtypes.bfloat16

B, CIN, COUT, L = 32, 256, 256, 4096
KTAPS, DIL, PAD = 7, 8, 28
LD = KTAPS * DIL  # 56
TOUT = L + 1  # 4097
NCORES = 8
BPC = B // NCORES  # 4

F = 256
NBIN = F // 2 + 1  # 129
V = F - LD + 1  # 201
SEGS = (TOUT + V - 1) // V  # 21
YL = SEGS * V  # 4221
LPAD = V * (SEGS - 1) + F  # 4276

XG = 3   # segs per xs load
DG = 4   # bins per dht load
NG = 8   # bins per negation batch
YG = 3   # segs per y store

_nc_cache = {}


def build_dense_kernel(weight: np.ndarray, P: np.ndarray) -> np.ndarray:
    """Scatter taps into dense [O, C, LD] kernel (fp32-exact vs reference)."""
    w = weight.astype(np.float32)
    pos = np.clip(P.astype(np.float32) + np.float32(LD // 2), np.float32(0.0), np.float32(LD - 1))
    lo = np.floor(pos)
    frac = pos - lo
    lo_i = lo.astype(np.int64)
    hi_i = np.minimum(lo_i + 1, LD - 1)
    O, C, K = w.shape
    oi = np.arange(O)[:, None, None]
    ci = np.arange(C)[None, :, None]
    D = np.zeros((O, C, LD), np.float32)
    np.add.at(D, (oi, ci, lo_i), w * (np.float32(1.0) - frac))
    np.add.at(D, (oi, ci, hi_i), w * frac)
    return D


def build_consts(D):
    """Wf [256,256], WI [256,V], Dh [129,2,C,O] (fp32; cast at use)."""
    t = np.arange(F)[:, None]
    k = np.arange(NBIN)[None, :]
    ang = 2 * np.pi * t * k / F
    Wf = np.concatenate([np.cos(ang), -np.sin(ang[:, 1:128])], axis=1)
    m = np.arange(V)[None, :]
    kk = np.arange(NBIN)[:, None]
    alpha = np.where((kk == 0) | (kk == NBIN - 1), 1.0, 2.0) / F
    angi = 2 * np.pi * kk * m / F
    WI = np.concatenate([alpha * np.cos(angi), -(alpha * np.sin(angi))[1:128]], axis=0)
    Kh = np.conj(np.fft.rfft(D, n=F, axis=2))  # [O,C,129]
    Dh = np.stack([Kh.real, Kh.imag], axis=0)  # [2,O,C,129]
    Dh = np.ascontiguousarray(np.transpose(Dh, (3, 0, 2, 1)))  # [129,2,C,O]
    return Wf.astype(np.float32), WI.astype(np.float32), Dh.astype(np.float32)


def build_nc(bpc=BPC):
    S = bpc * SEGS
    nc = bacc.Bacc("TRN2", target_bir_lowering=False, debug=False)
    # xt[b, tp, seg, tcn, c] -- per (b,tp) all segs contiguous
    xt_d = nc.dram_tensor("xt", [bpc, 128, SEGS, 2, CIN], DT.bfloat16, kind="ExternalInput").ap()
    # dh[k, cp, cb, ri, o] -- each bin one DRAM-contiguous 256KB block;
    # bins alternate the two HW-DGE queues (sync even / scalar odd)
    dh_d = nc.dram_tensor("dh", [NBIN, 128, 2, 2, COUT], DT.bfloat16, kind="ExternalInput").ap()
    wf_d = nc.dram_tensor("wf", [2, 128, 256], DT.bfloat16, kind="ExternalInput").ap()
    wi_d = nc.dram_tensor("wi", [2, 128, V], DT.bfloat16, kind="ExternalInput").ap()
    bias_d = nc.dram_tensor("bias", [128, 2], DT.float32, kind="ExternalInput").ap()
    # y[b, op, ot, t] bf16 -- host upcasts and reorders
    y_d = nc.dram_tensor("y", [bpc, 128, 2, YL], DT.bfloat16, kind="ExternalOutput").ap()

    with ExitStack() as ctx:
        tc = ctx.enter_context(tile.TileContext(nc))
        cpool = ctx.enter_context(tc.tile_pool(name="c", bufs=1))

        wft = cpool.tile([128, 2, 256], DT.bfloat16)
        wit = cpool.tile([128, 2, V], DT.bfloat16)
        biast = cpool.tile([128, 2], DT.float32)
        for tcn in range(2):
            nc.scalar.dma_start(wft[:, tcn, :], wf_d[tcn])
            nc.scalar.dma_start(wit[:, tcn, :], wi_d[tcn])
        nc.scalar.dma_start(biast[:], bias_d[:])

        # persistent SBUF stores
        xh = cpool.tile([128, 2, 256, S], DT.bfloat16, name="xh", tag="xh")
        ohA = cpool.tile([128, S, 256], DT.bfloat16, name="ohA", tag="ohA")
        ohB = cpool.tile([128, S, 256], DT.bfloat16, name="ohB", tag="ohB")

        # ---- forward DFT ----
        with tc.tile_pool(name="x", bufs=4) as xpool, \
             tc.tile_pool(name="psF", bufs=4, space="PSUM") as psF:
            qrr = 0
            for b in range(bpc):
                for g in range(SEGS // XG):
                    xs = xpool.tile([128, XG, 2, CIN], DT.bfloat16)
                    nc.sync.dma_start(xs[:], xt_d[b, :, g * XG : (g + 1) * XG])
                    for j in range(XG):
                        s = b * SEGS + g * XG + j
                        pf = psF.tile([128, 2, 256], DT.float32)
                        for cb in range(2):
                            for tcn in range(2):
                                nc.tensor.matmul(
                                    pf[:, cb, :],
                                    xs[:, j, tcn, cb * 128 : (cb + 1) * 128],
                                    wft[:, tcn, :],
                                    start=(tcn == 0),
                                    stop=(tcn == 1),
                                    skip_group_check=True,
                                )
                        # xs loads live on sync alone, so scalar is free
                        # to split drains with vector
                        if s % 2 == 0:
                            nc.vector.tensor_copy(xh[:, :, :, s], pf[:])
                        else:
                            nc.scalar.copy(xh[:, :, :, s], pf[:])

        # ---- pointwise complex multiply ----
        GRP = 2
        with tc.tile_pool(name="dh", bufs=8) as dhpool, \
             tc.tile_pool(name="ng", bufs=2) as ngpool, \
             tc.tile_pool(name="st", bufs=3) as stpool, \
             tc.tile_pool(name="psPW", bufs=4, space="PSUM") as psPW:
            stA = stB = None
            stA_rows = stB_rows = None

            def flush(st, rows, oh, eng):
                if st is None or not rows:
                    return
                # partition dim must stay outermost in SBUF APs, so scatter
                # row-by-row: dst [1, S, 256] <- src [S, 1, 256]
                for j, r in enumerate(rows):
                    eng.dma_start(oh[r : r + 1], st[:, j : j + 1, :])

            ng = None
            ng_k0 = -NG
            for k in range(NBIN):
                dht = dhpool.tile([128, 2, 2, COUT], DT.bfloat16)
                eng = nc.sync if k % 2 == 0 else nc.scalar
                eng.dma_start(dht[:], dh_d[k])
                has_im = 0 < k < NBIN - 1
                if has_im and k >= ng_k0 + NG:
                    ng_k0 = k
                    ng = ngpool.tile([128, 2, NG, S], DT.bfloat16)
                    kn = min(NG, NBIN - 1 - k)
                    for cb in range(2):
                        nc.vector.tensor_scalar_mul(
                            ng[:, cb, :kn, :], xh[:, cb, 128 + k : 128 + k + kn, :], -1.0
                        )
                ps = psPW.tile([S, 2, 256], DT.float32)
                # Re: Xr*Dr + (-Xi)*Di
                n_acc = 4 if has_im else 2
                idx = 0
                for cb in range(2):
                    nc.tensor.matmul(
                        ps[:, 0, :], xh[:, cb, k, :], dht[:, cb, 0, :],
                        start=(idx == 0), stop=(idx == n_acc - 1),
                        skip_group_check=True,
                    )
                    idx += 1
                if has_im:
                    for cb in range(2):
                        nc.tensor.matmul(
                            ps[:, 0, :], ng[:, cb, k - ng_k0, :], dht[:, cb, 1, :],
                            start=False, stop=(idx == n_acc - 1),
                            skip_group_check=True,
                        )
                        idx += 1
                    # Im: Xr*Di + Xi*Dr
                    idx = 0
                    for cb in range(2):
                        nc.tensor.matmul(
                            ps[:, 1, :], xh[:, cb, k, :], dht[:, cb, 1, :],
                            start=(idx == 0), stop=False,
                            skip_group_check=True,
                        )
                        idx += 1
                    for cb in range(2):
                        nc.tensor.matmul(
                            ps[:, 1, :], xh[:, cb, 128 + k, :], dht[:, cb, 0, :],
                            start=False, stop=(idx == 3),
                            skip_group_check=True,
                        )
                        idx += 1
                # stage Re row (ohA row k, or ohB row 0 for k=128) on scalar
                if k < 128:
                    if stA is None:
                        stA = stpool.tile([S, GRP, 256], DT.bfloat16)
                        stA_rows = []
                    nc.vector.tensor_copy(stA[:, len(stA_rows), :], ps[:, 0, :])
                    stA_rows.append(k)
                    if len(stA_rows) == GRP:
                        flush(stA, stA_rows, ohA, nc.gpsimd)
                        stA = None
                else:
                    stx = stpool.tile([S, 1, 256], DT.bfloat16)
                    nc.vector.tensor_copy(stx[:, 0, :], ps[:, 0, :])
                    flush(stx, [0], ohB, nc.gpsimd)
                # stage Im row (ohB row k) on vector
                if has_im:
                    if stB is None:
                        stB = stpool.tile([S, GRP, 256], DT.bfloat16)
                        stB_rows = []
                    nc.vector.tensor_copy(stB[:, len(stB_rows), :], ps[:, 1, :])
                    stB_rows.append(k)
                    if len(stB_rows) == GRP:
                        flush(stB, stB_rows, ohB, nc.gpsimd)
                        stB = None
            flush(stA, stA_rows, ohA, nc.gpsimd)
            flush(stB, stB_rows, ohB, nc.gpsimd)

        # ---- inverse DFT + bias ----
        with tc.tile_pool(name="y", bufs=3) as ypool, \
             tc.tile_pool(name="psI", bufs=4, space="PSUM") as psI:
            ys = None
            qrr = 0
            for s in range(S):
                b, i = divmod(s, SEGS)
                j = i % YG
                if j == 0:
                    ys = ypool.tile([128, 2, YG, V], DT.bfloat16)
                pv = psI.tile([128, 2, V], DT.float32)
                for ot in range(2):
                    nc.tensor.matmul(
                        pv[:, ot, :], ohA[:, s, ot * 128 : (ot + 1) * 128], wit[:, 0, :],
                        start=True, stop=False, skip_group_check=True,
                    )
                    nc.tensor.matmul(
                        pv[:, ot, :], ohB[:, s, ot * 128 : (ot + 1) * 128], wit[:, 1, :],
                        start=False, stop=True, skip_group_check=True,
                    )
                nc.vector.tensor_scalar_add(
                    ys[:, 0, j, :], pv[:, 0, :], biast[:, 0:1]
                )
                nc.scalar.add(ys[:, 1, j, :], pv[:, 1, :], biast[:, 1:2])
                if j == YG - 1:
                    eng = nc.scalar if qrr % 2 == 0 else nc.sync
                    qrr += 1
                    eng.dma_start(
                        y_d[b, :, :, (i - YG + 1) * V : (i + 1) * V],
                        ys[:],
                    )

    nc.compile()
    return nc


def host_inputs(input, weight, P, bias):
    """Host-side staging: xt segments (transposed, bf16) + spectra consts."""
    D = build_dense_kernel(weight, P)
    Wf, WI, Dh = build_consts(D)
    wf = np.ascontiguousarray(Wf.reshape(2, 128, 256)).astype(BF)
    wi = np.ascontiguousarray(WI.reshape(2, 128, V)).astype(BF)
    # Dh [129, 2ri, C, O] -> dh[cp, k, cb, ri, o]
    dh = np.ascontiguousarray(
        Dh.reshape(NBIN, 2, 2, 128, COUT).transpose(0, 3, 2, 1, 4)
    ).astype(BF)
    bias2 = np.ascontiguousarray(np.asarray(bias, np.float32).reshape(2, 128).T)
    xpad = np.zeros((input.shape[0], CIN, LPAD), np.float32)
    xpad[:, :, PAD : PAD + L] = input
    xpad = xpad.astype(BF)
    idx = V * np.arange(SEGS)[:, None] + np.arange(F)[None, :]
    segs = xpad[:, :, idx]  # [B, C, SEGS, F]
    # [B, SEGS, F, C] -> [B, SEGS, tcn, tp, C] -> [B, tp, SEGS, tcn, C]
    xt = segs.transpose(0, 2, 3, 1).reshape(input.shape[0], SEGS, 2, 128, CIN)
    xt = np.ascontiguousarray(xt.transpose(0, 3, 1, 2, 4))
    return xt, dh, wf, wi, bias2


def make_in_maps(inputs):
    xt, dh, wf, wi, bias2 = host_inputs(
        np.ascontiguousarray(inputs["input"], np.float32),
        inputs["weight"],
        inputs["P"],
        inputs["bias"],
    )
    return [
        {
            "xt": np.ascontiguousarray(xt[i * BPC : (i + 1) * BPC]),
            "dh": dh,
            "wf": wf,
            "wi": wi,
            "bias": bias2,
        }
        for i in range(NCORES)
    ]


def kernel(input, weight, P, bias):
    if "nc" not in _nc_cache:
        _nc_cache["nc"] = build_nc()
    nc = _nc_cache["nc"]
    in_maps = make_in_maps(
        {"input": input, "weight": weight, "P": P, "bias": bias}
    )
    res = run_bass_kernel_spmd(nc, in_maps, core_ids=list(range(NCORES)))
    out = np.concatenate(
        [
            np.asarray(r["y"])
            .astype(np.float32)
            .transpose(0, 2, 1, 3)
            .reshape(BPC, COUT, YL)
            for r in res.results
        ],
        axis=0,
    )
    return np.ascontiguousarray(out[:, :, :TOUT])


# revision 17
# speedup vs baseline: 1.2918x; 1.0043x over previous
"""Dcls1d via overlap-save rFFT conv on 8 Trainium2 NeuronCores.

F=256 overlap-save FFT convolution, data-parallel over batch (4/core):
  fwd:  per segment (21/batch, V=201 valid outs), DFT as 2-chain matmuls
        -> x_hat[c, bins] in SBUF (bf16). xs loads grouped 3 segs/DMA,
        alternating the two HW-DGE queues (sync/scalar).
  pw:   per bin k, complex pointwise mult-accumulate over c as matmuls;
        D_hat streamed from DRAM in 4-bin groups with 8KB-contiguous
        descriptors, alternating both HW queues. -Xi negations batched
        8 bins at a time on vector. psum drains: Re->scalar, Im->vector.
        Corner-turn flushes (psum-partition s -> oh-partition bin) as
        per-row SBUF->SBUF DMAs split between gpsimd and sync.
  inv:  per segment, irFFT matmuls + bias, staged 3 segs then one bf16
        DMA out per group, alternating queues (y written bf16; host
        upcasts -- well inside the 2e-2 tolerance).

Host precomputes D_hat = conj(rfft(D_dense, 256)) in bf16.
"""

import numpy as np
import ml_dtypes
from contextlib import ExitStack

import concourse.bacc as bacc
import concourse.mybir as mybir
import concourse.tile as tile
from concourse.bass_utils import run_bass_kernel_spmd

DT = mybir.dt
BF = ml_dtypes.bfloat16

B, CIN, COUT, L = 32, 256, 256, 4096
KTAPS, DIL, PAD = 7, 8, 28
LD = KTAPS * DIL  # 56
TOUT = L + 1  # 4097
NCORES = 8
BPC = B // NCORES  # 4

F = 256
NBIN = F // 2 + 1  # 129
V = F - LD + 1  # 201
SEGS = (TOUT + V - 1) // V  # 21
YL = SEGS * V  # 4221
LPAD = V * (SEGS - 1) + F  # 4276

XG = 3   # segs per xs load
DG = 4   # bins per dht load
NG = 8   # bins per negation batch
YG = 3   # segs per y store

_nc_cache = {}


def build_dense_kernel(weight: np.ndarray, P: np.ndarray) -> np.ndarray:
    """Scatter taps into dense [O, C, LD] kernel (fp32-exact vs reference)."""
    w = weight.astype(np.float32)
    pos = np.clip(P.astype(np.float32) + np.float32(LD // 2), np.float32(0.0), np.float32(LD - 1))
    lo = np.floor(pos)
    frac = pos - lo
    lo_i = lo.astype(np.int64)
    hi_i = np.minimum(lo_i + 1, LD - 1)
    O, C, K = w.shape
    oi = np.arange(O)[:, None, None]
    ci = np.arange(C)[None, :, None]
    D = np.zeros((O, C, LD), np.float32)
    np.add.at(D, (oi, ci, lo_i), w * (np.float32(1.0) - frac))
    np.add.at(D, (oi, ci, hi_i), w * frac)
    return D


def build_consts(D):
    """Wf [256,256], WI [256,V], Dh [129,2,C,O] (fp32; cast at use)."""
    t = np.arange(F)[:, None]
    k = np.arange(NBIN)[None, :]
    ang = 2 * np.pi * t * k / F
    Wf = np.concatenate([np.cos(ang), -np.sin(ang[:, 1:128])], axis=1)
    m = np.arange(V)[None, :]
    kk = np.arange(NBIN)[:, None]
    alpha = np.where((kk == 0) | (kk == NBIN - 1), 1.0, 2.0) / F
    angi = 2 * np.pi * kk * m / F
    WI = np.concatenate([alpha * np.cos(angi), -(alpha * np.sin(angi))[1:128]], axis=0)
    Kh = np.conj(np.fft.rfft(D, n=F, axis=2))  # [O,C,129]
    Dh = np.stack([Kh.real, Kh.imag], axis=0)  # [2,O,C,129]
    Dh = np.ascontiguousarray(np.transpose(Dh, (3, 0, 2, 1)))  # [129,2,C,O]
    return Wf.astype(np.float32), WI.astype(np.float32), Dh.astype(np.float32)


def build_nc(bpc=BPC):
    S = bpc * SEGS
    nc = bacc.Bacc("TRN2", target_bir_lowering=False, debug=False)
    # xt[b, tp, seg, tcn, c] -- per (b,tp) all segs contiguous
    xt_d = nc.dram_tensor("xt", [bpc, 128, SEGS, 2, CIN], DT.bfloat16, kind="ExternalInput").ap()
    # dh[kpair, cp, kin, cb, ri, o] -- each bin-PAIR one DRAM-contiguous
    # 512KB block with 4KB-contiguous per-partition runs (the profile shows
    # this shape is what sustains ~117GB/s reads on one HW queue); bin 129
    # is zero padding. All pairs stream on sync, which stays a pure issuer.
    dh_d = nc.dram_tensor("dh", [(NBIN + 1) // 2, 128, 2, 2, 2, COUT], DT.bfloat16, kind="ExternalInput").ap()
    wf_d = nc.dram_tensor("wf", [2, 128, 256], DT.bfloat16, kind="ExternalInput").ap()
    wi_d = nc.dram_tensor("wi", [2, 128, V], DT.bfloat16, kind="ExternalInput").ap()
    bias_d = nc.dram_tensor("bias", [128, 2], DT.float32, kind="ExternalInput").ap()
    # y[b, op, ot, t] bf16 -- host upcasts and reorders
    y_d = nc.dram_tensor("y", [bpc, 128, 2, YL], DT.bfloat16, kind="ExternalOutput").ap()

    with ExitStack() as ctx:
        tc = ctx.enter_context(tile.TileContext(nc))
        cpool = ctx.enter_context(tc.tile_pool(name="c", bufs=1))

        wft = cpool.tile([128, 2, 256], DT.bfloat16)
        wit = cpool.tile([128, 2, V], DT.bfloat16)
        biast = cpool.tile([128, 2], DT.float32)
        for tcn in range(2):
            nc.scalar.dma_start(wft[:, tcn, :], wf_d[tcn])
            nc.scalar.dma_start(wit[:, tcn, :], wi_d[tcn])
        nc.scalar.dma_start(biast[:], bias_d[:])

        # persistent SBUF stores
        xh = cpool.tile([128, 2, 256, S], DT.bfloat16, name="xh", tag="xh")
        ohA = cpool.tile([128, S, 256], DT.bfloat16, name="ohA", tag="ohA")
        ohB = cpool.tile([128, S, 256], DT.bfloat16, name="ohB", tag="ohB")

        # ---- forward DFT ----
        with tc.tile_pool(name="x", bufs=4) as xpool, \
             tc.tile_pool(name="psF", bufs=4, space="PSUM") as psF:
            qrr = 0
            for b in range(bpc):
                for g in range(SEGS // XG):
                    xs = xpool.tile([128, XG, 2, CIN], DT.bfloat16)
                    nc.sync.dma_start(xs[:], xt_d[b, :, g * XG : (g + 1) * XG])
                    for j in range(XG):
                        s = b * SEGS + g * XG + j
                        pf = psF.tile([128, 2, 256], DT.float32)
                        for cb in range(2):
                            for tcn in range(2):
                                nc.tensor.matmul(
                                    pf[:, cb, :],
                                    xs[:, j, tcn, cb * 128 : (cb + 1) * 128],
                                    wft[:, tcn, :],
                                    start=(tcn == 0),
                                    stop=(tcn == 1),
                                    skip_group_check=True,
                                )
                        # xs loads live on sync alone, so scalar is free
                        # to split drains with vector
                        if s % 2 == 0:
                            nc.vector.tensor_copy(xh[:, :, :, s], pf[:])
                        else:
                            nc.scalar.copy(xh[:, :, :, s], pf[:])

        # ---- pointwise complex multiply ----
        GRP = 2
        with tc.tile_pool(name="dh", bufs=6) as dhpool, \
             tc.tile_pool(name="ng", bufs=2) as ngpool, \
             tc.tile_pool(name="st", bufs=3) as stpool, \
             tc.tile_pool(name="psPW", bufs=4, space="PSUM") as psPW:
            stA = stB = None
            stA_rows = stB_rows = None

            def flush(st, rows, oh, eng):
                if st is None or not rows:
                    return
                # partition dim must stay outermost in SBUF APs, so scatter
                # row-by-row: dst [1, S, 256] <- src [S, 1, 256]
                for j, r in enumerate(rows):
                    eng.dma_start(oh[r : r + 1], st[:, j : j + 1, :])

            ng = None
            ng_k0 = -NG
            dht = None
            for k in range(NBIN):
                jd = k % 2
                if jd == 0:
                    dht = dhpool.tile([128, 2, 2, 2, COUT], DT.bfloat16)
                    nc.sync.dma_start(dht[:], dh_d[k // 2])
                has_im = 0 < k < NBIN - 1
                if has_im and k >= ng_k0 + NG:
                    ng_k0 = k
                    ng = ngpool.tile([128, 2, NG, S], DT.bfloat16)
                    kn = min(NG, NBIN - 1 - k)
                    for cb in range(2):
                        nc.vector.tensor_scalar_mul(
                            ng[:, cb, :kn, :], xh[:, cb, 128 + k : 128 + k + kn, :], -1.0
                        )
                ps = psPW.tile([S, 2, 256], DT.float32)
                # Re: Xr*Dr + (-Xi)*Di
                n_acc = 4 if has_im else 2
                idx = 0
                for cb in range(2):
                    nc.tensor.matmul(
                        ps[:, 0, :], xh[:, cb, k, :], dht[:, jd, cb, 0, :],
                        start=(idx == 0), stop=(idx == n_acc - 1),
                        skip_group_check=True,
                    )
                    idx += 1
                if has_im:
                    for cb in range(2):
                        nc.tensor.matmul(
                            ps[:, 0, :], ng[:, cb, k - ng_k0, :], dht[:, jd, cb, 1, :],
                            start=False, stop=(idx == n_acc - 1),
                            skip_group_check=True,
                        )
                        idx += 1
                    # Im: Xr*Di + Xi*Dr
                    idx = 0
                    for cb in range(2):
                        nc.tensor.matmul(
                            ps[:, 1, :], xh[:, cb, k, :], dht[:, jd, cb, 1, :],
                            start=(idx == 0), stop=False,
                            skip_group_check=True,
                        )
                        idx += 1
                    for cb in range(2):
                        nc.tensor.matmul(
                            ps[:, 1, :], xh[:, cb, 128 + k, :], dht[:, jd, cb, 0, :],
                            start=False, stop=(idx == 3),
                            skip_group_check=True,
                        )
                        idx += 1
                # stage Re row (ohA row k, or ohB row 0 for k=128) on scalar
                if k < 128:
                    if stA is None:
                        stA = stpool.tile([S, GRP, 256], DT.bfloat16)
                        stA_rows = []
                    nc.vector.tensor_copy(stA[:, len(stA_rows), :], ps[:, 0, :])
                    stA_rows.append(k)
                    if len(stA_rows) == GRP:
                        flush(stA, stA_rows, ohA, nc.gpsimd)
                        stA = None
                else:
                    stx = stpool.tile([S, 1, 256], DT.bfloat16)
                    nc.vector.tensor_copy(stx[:, 0, :], ps[:, 0, :])
                    flush(stx, [0], ohB, nc.gpsimd)
                # stage Im row (ohB row k) on vector
                if has_im:
                    if stB is None:
                        stB = stpool.tile([S, GRP, 256], DT.bfloat16)
                        stB_rows = []
                    nc.vector.tensor_copy(stB[:, len(stB_rows), :], ps[:, 1, :])
                    stB_rows.append(k)
                    if len(stB_rows) == GRP:
                        flush(stB, stB_rows, ohB, nc.gpsimd)
                        stB = None
            flush(stA, stA_rows, ohA, nc.gpsimd)
            flush(stB, stB_rows, ohB, nc.gpsimd)

        # ---- inverse DFT + bias ----
        with tc.tile_pool(name="y", bufs=3) as ypool, \
             tc.tile_pool(name="psI", bufs=4, space="PSUM") as psI:
            ys = None
            qrr = 0
            for s in range(S):
                b, i = divmod(s, SEGS)
                j = i % YG
                if j == 0:
                    ys = ypool.tile([128, 2, YG, V], DT.bfloat16)
                pv = psI.tile([128, 2, V], DT.float32)
                for ot in range(2):
                    nc.tensor.matmul(
                        pv[:, ot, :], ohA[:, s, ot * 128 : (ot + 1) * 128], wit[:, 0, :],
                        start=True, stop=False, skip_group_check=True,
                    )
                    nc.tensor.matmul(
                        pv[:, ot, :], ohB[:, s, ot * 128 : (ot + 1) * 128], wit[:, 1, :],
                        start=False, stop=True, skip_group_check=True,
                    )
                nc.vector.tensor_scalar_add(
                    ys[:, 0, j, :], pv[:, 0, :], biast[:, 0:1]
                )
                nc.scalar.add(ys[:, 1, j, :], pv[:, 1, :], biast[:, 1:2])
                if j == YG - 1:
                    eng = nc.scalar if qrr % 2 == 0 else nc.sync
                    qrr += 1
                    eng.dma_start(
                        y_d[b, :, :, (i - YG + 1) * V : (i + 1) * V],
                        ys[:],
                    )

    nc.compile()
    return nc


def host_inputs(input, weight, P, bias):
    """Host-side staging: xt segments (transposed, bf16) + spectra consts."""
    D = build_dense_kernel(weight, P)
    Wf, WI, Dh = build_consts(D)
    wf = np.ascontiguousarray(Wf.reshape(2, 128, 256)).astype(BF)
    wi = np.ascontiguousarray(WI.reshape(2, 128, V)).astype(BF)
    # Dh [129, 2ri, C, O] -> dh[k, cp, cb, ri, o] -> pad to 130 bins ->
    # pair-major [kpair, cp, kin, cb, ri, o]
    dh1 = Dh.reshape(NBIN, 2, 2, 128, COUT).transpose(0, 3, 2, 1, 4)
    dhp = np.zeros((NBIN + 1, 128, 2, 2, COUT), np.float32)
    dhp[:NBIN] = dh1
    dh = np.ascontiguousarray(
        dhp.reshape((NBIN + 1) // 2, 2, 128, 2, 2, COUT).transpose(0, 2, 1, 3, 4, 5)
    ).astype(BF)
    bias2 = np.ascontiguousarray(np.asarray(bias, np.float32).reshape(2, 128).T)
    xpad = np.zeros((input.shape[0], CIN, LPAD), np.float32)
    xpad[:, :, PAD : PAD + L] = input
    xpad = xpad.astype(BF)
    idx = V * np.arange(SEGS)[:, None] + np.arange(F)[None, :]
    segs = xpad[:, :, idx]  # [B, C, SEGS, F]
    # [B, SEGS, F, C] -> [B, SEGS, tcn, tp, C] -> [B, tp, SEGS, tcn, C]
    xt = segs.transpose(0, 2, 3, 1).reshape(input.shape[0], SEGS, 2, 128, CIN)
    xt = np.ascontiguousarray(xt.transpose(0, 3, 1, 2, 4))
    return xt, dh, wf, wi, bias2


def make_in_maps(inputs):
    xt, dh, wf, wi, bias2 = host_inputs(
        np.ascontiguousarray(inputs["input"], np.float32),
        inputs["weight"],
        inputs["P"],
        inputs["bias"],
    )
    return [
        {
            "xt": np.ascontiguousarray(xt[i * BPC : (i + 1) * BPC]),
            "dh": dh,
            "wf": wf,
            "wi": wi,
            "bias": bias2,
        }
        for i in range(NCORES)
    ]


def kernel(input, weight, P, bias):
    if "nc" not in _nc_cache:
        _nc_cache["nc"] = build_nc()
    nc = _nc_cache["nc"]
    in_maps = make_in_maps(
        {"input": input, "weight": weight, "P": P, "bias": bias}
    )
    res = run_bass_kernel_spmd(nc, in_maps, core_ids=list(range(NCORES)))
    out = np.concatenate(
        [
            np.asarray(r["y"])
            .astype(np.float32)
            .transpose(0, 2, 1, 3)
            .reshape(BPC, COUT, YL)
            for r in res.results
        ],
        axis=0,
    )
    return np.ascontiguousarray(out[:, :, :TOUT])


# revision 18
# speedup vs baseline: 1.3582x; 1.0514x over previous
"""Dcls1d via overlap-save rFFT conv on 8 Trainium2 NeuronCores.

F=256 overlap-save FFT convolution, data-parallel over batch (4/core):
  fwd:  per segment (21/batch, V=201 valid outs), DFT as 2-chain matmuls
        -> x_hat[c, bins] in SBUF (bf16). xs loads grouped 3 segs/DMA,
        alternating the two HW-DGE queues (sync/scalar).
  pw:   per bin k, complex pointwise mult-accumulate over c as matmuls;
        D_hat streamed from DRAM in 4-bin groups with 8KB-contiguous
        descriptors, alternating both HW queues. -Xi negations batched
        8 bins at a time on vector. psum drains: Re->scalar, Im->vector.
        Corner-turn flushes (psum-partition s -> oh-partition bin) as
        per-row SBUF->SBUF DMAs split between gpsimd and sync.
  inv:  per segment, irFFT matmuls + bias, staged 3 segs then one bf16
        DMA out per group, alternating queues (y written bf16; host
        upcasts -- well inside the 2e-2 tolerance).

Host precomputes D_hat = conj(rfft(D_dense, 256)) in bf16.
"""

import numpy as np
import ml_dtypes
from contextlib import ExitStack

import concourse.bacc as bacc
import concourse.mybir as mybir
import concourse.tile as tile
from concourse.bass_utils import run_bass_kernel_spmd

DT = mybir.dt
BF = ml_dtypes.bfloat16

B, CIN, COUT, L = 32, 256, 256, 4096
KTAPS, DIL, PAD = 7, 8, 28
LD = KTAPS * DIL  # 56
TOUT = L + 1  # 4097
NCORES = 8
BPC = B // NCORES  # 4

F = 256
NBIN = F // 2 + 1  # 129
V = F - LD + 1  # 201
SEGS = (TOUT + V - 1) // V  # 21
YL = SEGS * V  # 4221
LPAD = V * (SEGS - 1) + F  # 4276

XG = 3   # segs per xs load
DG = 4   # bins per dht load
NG = 8   # bins per negation batch
YG = 3   # segs per y store

_nc_cache = {}


def build_dense_kernel(weight: np.ndarray, P: np.ndarray) -> np.ndarray:
    """Scatter taps into dense [O, C, LD] kernel (fp32-exact vs reference)."""
    w = weight.astype(np.float32)
    pos = np.clip(P.astype(np.float32) + np.float32(LD // 2), np.float32(0.0), np.float32(LD - 1))
    lo = np.floor(pos)
    frac = pos - lo
    lo_i = lo.astype(np.int64)
    hi_i = np.minimum(lo_i + 1, LD - 1)
    O, C, K = w.shape
    oi = np.arange(O)[:, None, None]
    ci = np.arange(C)[None, :, None]
    D = np.zeros((O, C, LD), np.float32)
    np.add.at(D, (oi, ci, lo_i), w * (np.float32(1.0) - frac))
    np.add.at(D, (oi, ci, hi_i), w * frac)
    return D


def build_consts(D):
    """Wf [256,256], WI [256,V], Dh [129,2,C,O] (fp32; cast at use)."""
    t = np.arange(F)[:, None]
    k = np.arange(NBIN)[None, :]
    ang = 2 * np.pi * t * k / F
    Wf = np.concatenate([np.cos(ang), -np.sin(ang[:, 1:128])], axis=1)
    m = np.arange(V)[None, :]
    kk = np.arange(NBIN)[:, None]
    alpha = np.where((kk == 0) | (kk == NBIN - 1), 1.0, 2.0) / F
    angi = 2 * np.pi * kk * m / F
    WI = np.concatenate([alpha * np.cos(angi), -(alpha * np.sin(angi))[1:128]], axis=0)
    Kh = np.conj(np.fft.rfft(D, n=F, axis=2))  # [O,C,129]
    Dh = np.stack([Kh.real, Kh.imag], axis=0)  # [2,O,C,129]
    Dh = np.ascontiguousarray(np.transpose(Dh, (3, 0, 2, 1)))  # [129,2,C,O]
    return Wf.astype(np.float32), WI.astype(np.float32), Dh.astype(np.float32)


def build_nc(bpc=BPC):
    S = bpc * SEGS
    nc = bacc.Bacc("TRN2", target_bir_lowering=False, debug=False)
    # xt[b, tp, seg, tcn, c] -- per (b,tp) all segs contiguous
    xt_d = nc.dram_tensor("xt", [bpc, 128, SEGS, 2, CIN], DT.bfloat16, kind="ExternalInput").ap()
    # dh[kpair, cp, kin, cb, ri, o] -- each bin-PAIR one DRAM-contiguous
    # 512KB block with 4KB-contiguous per-partition runs (the profile shows
    # this shape is what sustains ~117GB/s reads on one HW queue); bin 129
    # is zero padding. All pairs stream on sync, which stays a pure issuer.
    dh_d = nc.dram_tensor("dh", [(NBIN + 1) // 2, 128, 2, 2, 2, COUT], DT.bfloat16, kind="ExternalInput").ap()
    wf_d = nc.dram_tensor("wf", [2, 128, 256], DT.bfloat16, kind="ExternalInput").ap()
    wi_d = nc.dram_tensor("wi", [2, 128, V], DT.bfloat16, kind="ExternalInput").ap()
    bias_d = nc.dram_tensor("bias", [128, 2], DT.float32, kind="ExternalInput").ap()
    # y[b, op, ot, t] bf16 -- host upcasts and reorders
    y_d = nc.dram_tensor("y", [bpc, 128, 2, YL], DT.bfloat16, kind="ExternalOutput").ap()

    with ExitStack() as ctx:
        tc = ctx.enter_context(tile.TileContext(nc))
        cpool = ctx.enter_context(tc.tile_pool(name="c", bufs=1))

        wft = cpool.tile([128, 2, 256], DT.bfloat16)
        wit = cpool.tile([128, 2, V], DT.bfloat16)
        biast = cpool.tile([128, 2], DT.float32)
        for tcn in range(2):
            nc.scalar.dma_start(wft[:, tcn, :], wf_d[tcn])
            nc.scalar.dma_start(wit[:, tcn, :], wi_d[tcn])
        nc.scalar.dma_start(biast[:], bias_d[:])

        # persistent SBUF stores
        xh = cpool.tile([128, 2, 256, S], DT.bfloat16, name="xh", tag="xh")
        ohA = cpool.tile([128, S, 256], DT.bfloat16, name="ohA", tag="ohA")
        ohB = cpool.tile([128, S, 256], DT.bfloat16, name="ohB", tag="ohB")

        # ---- forward DFT ----
        with tc.tile_pool(name="x", bufs=4) as xpool, \
             tc.tile_pool(name="psF", bufs=4, space="PSUM") as psF:
            qrr = 0
            for b in range(bpc):
                for g in range(SEGS // XG):
                    xs = xpool.tile([128, XG, 2, CIN], DT.bfloat16)
                    nc.sync.dma_start(xs[:], xt_d[b, :, g * XG : (g + 1) * XG])
                    for j in range(XG):
                        s = b * SEGS + g * XG + j
                        pf = psF.tile([128, 2, 256], DT.float32)
                        for cb in range(2):
                            for tcn in range(2):
                                nc.tensor.matmul(
                                    pf[:, cb, :],
                                    xs[:, j, tcn, cb * 128 : (cb + 1) * 128],
                                    wft[:, tcn, :],
                                    start=(tcn == 0),
                                    stop=(tcn == 1),
                                    skip_group_check=True,
                                )
                        # xs loads live on sync alone, so scalar is free
                        # to split drains with vector
                        if s % 2 == 0:
                            nc.vector.tensor_copy(xh[:, :, :, s], pf[:])
                        else:
                            nc.scalar.copy(xh[:, :, :, s], pf[:])

        # ---- pointwise complex multiply ----
        GRP = 2
        with tc.tile_pool(name="dh", bufs=6) as dhpool, \
             tc.tile_pool(name="ng", bufs=2) as ngpool, \
             tc.tile_pool(name="st", bufs=3) as stpool, \
             tc.tile_pool(name="psPW", bufs=4, space="PSUM") as psPW:
            stA = stB = None
            stA_rows = stB_rows = None
            flush_rr = [0]
            # 3:1 scalar:gpsimd -- the scalar HW-DGE queue moves these
            # corner-turn rows ~4x faster than gpsimd's software DGE, and
            # scalar has no other pw duties (dht prefetch lives on sync)
            flush_engs = [nc.scalar, nc.scalar, nc.scalar, nc.gpsimd]

            def flush(st, rows, oh, eng=None):
                if st is None or not rows:
                    return
                # partition dim must stay outermost in SBUF APs, so scatter
                # row-by-row: dst [1, S, 256] <- src [S, 1, 256]
                for j, r in enumerate(rows):
                    e = flush_engs[flush_rr[0] % len(flush_engs)]
                    flush_rr[0] += 1
                    e.dma_start(oh[r : r + 1], st[:, j : j + 1, :])

            ng = None
            ng_k0 = -NG
            dht = None
            for k in range(NBIN):
                jd = k % 2
                if jd == 0:
                    dht = dhpool.tile([128, 2, 2, 2, COUT], DT.bfloat16)
                    nc.sync.dma_start(dht[:], dh_d[k // 2])
                has_im = 0 < k < NBIN - 1
                if has_im and k >= ng_k0 + NG:
                    ng_k0 = k
                    ng = ngpool.tile([128, 2, NG, S], DT.bfloat16)
                    kn = min(NG, NBIN - 1 - k)
                    for cb in range(2):
                        nc.vector.tensor_scalar_mul(
                            ng[:, cb, :kn, :], xh[:, cb, 128 + k : 128 + k + kn, :], -1.0
                        )
                ps = psPW.tile([S, 2, 256], DT.float32)
                # Re: Xr*Dr + (-Xi)*Di
                n_acc = 4 if has_im else 2
                idx = 0
                for cb in range(2):
                    nc.tensor.matmul(
                        ps[:, 0, :], xh[:, cb, k, :], dht[:, jd, cb, 0, :],
                        start=(idx == 0), stop=(idx == n_acc - 1),
                        skip_group_check=True,
                    )
                    idx += 1
                if has_im:
                    for cb in range(2):
                        nc.tensor.matmul(
                            ps[:, 0, :], ng[:, cb, k - ng_k0, :], dht[:, jd, cb, 1, :],
                            start=False, stop=(idx == n_acc - 1),
                            skip_group_check=True,
                        )
                        idx += 1
                    # Im: Xr*Di + Xi*Dr
                    idx = 0
                    for cb in range(2):
                        nc.tensor.matmul(
                            ps[:, 1, :], xh[:, cb, k, :], dht[:, jd, cb, 1, :],
                            start=(idx == 0), stop=False,
                            skip_group_check=True,
                        )
                        idx += 1
                    for cb in range(2):
                        nc.tensor.matmul(
                            ps[:, 1, :], xh[:, cb, 128 + k, :], dht[:, jd, cb, 0, :],
                            start=False, stop=(idx == 3),
                            skip_group_check=True,
                        )
                        idx += 1
                # stage Re row (ohA row k, or ohB row 0 for k=128) on scalar
                if k < 128:
                    if stA is None:
                        stA = stpool.tile([S, GRP, 256], DT.bfloat16)
                        stA_rows = []
                    nc.vector.tensor_copy(stA[:, len(stA_rows), :], ps[:, 0, :])
                    stA_rows.append(k)
                    if len(stA_rows) == GRP:
                        flush(stA, stA_rows, ohA, nc.gpsimd)
                        stA = None
                else:
                    stx = stpool.tile([S, 1, 256], DT.bfloat16)
                    nc.vector.tensor_copy(stx[:, 0, :], ps[:, 0, :])
                    flush(stx, [0], ohB, nc.gpsimd)
                # stage Im row (ohB row k) on vector
                if has_im:
                    if stB is None:
                        stB = stpool.tile([S, GRP, 256], DT.bfloat16)
                        stB_rows = []
                    nc.vector.tensor_copy(stB[:, len(stB_rows), :], ps[:, 1, :])
                    stB_rows.append(k)
                    if len(stB_rows) == GRP:
                        flush(stB, stB_rows, ohB, nc.gpsimd)
                        stB = None
            flush(stA, stA_rows, ohA, nc.gpsimd)
            flush(stB, stB_rows, ohB, nc.gpsimd)

        # ---- inverse DFT + bias ----
        with tc.tile_pool(name="y", bufs=3) as ypool, \
             tc.tile_pool(name="psI", bufs=4, space="PSUM") as psI:
            ys = None
            qrr = 0
            for s in range(S):
                b, i = divmod(s, SEGS)
                j = i % YG
                if j == 0:
                    ys = ypool.tile([128, 2, YG, V], DT.bfloat16)
                pv = psI.tile([128, 2, V], DT.float32)
                for ot in range(2):
                    nc.tensor.matmul(
                        pv[:, ot, :], ohA[:, s, ot * 128 : (ot + 1) * 128], wit[:, 0, :],
                        start=True, stop=False, skip_group_check=True,
                    )
                    nc.tensor.matmul(
                        pv[:, ot, :], ohB[:, s, ot * 128 : (ot + 1) * 128], wit[:, 1, :],
                        start=False, stop=True, skip_group_check=True,
                    )
                nc.vector.tensor_scalar_add(
                    ys[:, 0, j, :], pv[:, 0, :], biast[:, 0:1]
                )
                nc.scalar.add(ys[:, 1, j, :], pv[:, 1, :], biast[:, 1:2])
                if j == YG - 1:
                    eng = nc.scalar if qrr % 2 == 0 else nc.sync
                    qrr += 1
                    eng.dma_start(
                        y_d[b, :, :, (i - YG + 1) * V : (i + 1) * V],
                        ys[:],
                    )

    nc.compile()
    return nc


def host_inputs(input, weight, P, bias):
    """Host-side staging: xt segments (transposed, bf16) + spectra consts."""
    D = build_dense_kernel(weight, P)
    Wf, WI, Dh = build_consts(D)
    wf = np.ascontiguousarray(Wf.reshape(2, 128, 256)).astype(BF)
    wi = np.ascontiguousarray(WI.reshape(2, 128, V)).astype(BF)
    # Dh [129, 2ri, C, O] -> dh[k, cp, cb, ri, o] -> pad to 130 bins ->
    # pair-major [kpair, cp, kin, cb, ri, o]
    dh1 = Dh.reshape(NBIN, 2, 2, 128, COUT).transpose(0, 3, 2, 1, 4)
    dhp = np.zeros((NBIN + 1, 128, 2, 2, COUT), np.float32)
    dhp[:NBIN] = dh1
    dh = np.ascontiguousarray(
        dhp.reshape((NBIN + 1) // 2, 2, 128, 2, 2, COUT).transpose(0, 2, 1, 3, 4, 5)
    ).astype(BF)
    bias2 = np.ascontiguousarray(np.asarray(bias, np.float32).reshape(2, 128).T)
    xpad = np.zeros((input.shape[0], CIN, LPAD), np.float32)
    xpad[:, :, PAD : PAD + L] = input
    xpad = xpad.astype(BF)
    idx = V * np.arange(SEGS)[:, None] + np.arange(F)[None, :]
    segs = xpad[:, :, idx]  # [B, C, SEGS, F]
    # [B, SEGS, F, C] -> [B, SEGS, tcn, tp, C] -> [B, tp, SEGS, tcn, C]
    xt = segs.transpose(0, 2, 3, 1).reshape(input.shape[0], SEGS, 2, 128, CIN)
    xt = np.ascontiguousarray(xt.transpose(0, 3, 1, 2, 4))
    return xt, dh, wf, wi, bias2


def make_in_maps(inputs):
    xt, dh, wf, wi, bias2 = host_inputs(
        np.ascontiguousarray(inputs["input"], np.float32),
        inputs["weight"],
        inputs["P"],
        inputs["bias"],
    )
    return [
        {
            "xt": np.ascontiguousarray(xt[i * BPC : (i + 1) * BPC]),
            "dh": dh,
            "wf": wf,
            "wi": wi,
            "bias": bias2,
        }
        for i in range(NCORES)
    ]


def kernel(input, weight, P, bias):
    if "nc" not in _nc_cache:
        _nc_cache["nc"] = build_nc()
    nc = _nc_cache["nc"]
    in_maps = make_in_maps(
        {"input": input, "weight": weight, "P": P, "bias": bias}
    )
    res = run_bass_kernel_spmd(nc, in_maps, core_ids=list(range(NCORES)))
    out = np.concatenate(
        [
            np.asarray(r["y"])
            .astype(np.float32)
            .transpose(0, 2, 1, 3)
            .reshape(BPC, COUT, YL)
            for r in res.results
        ],
        axis=0,
    )
    return np.ascontiguousarray(out[:, :, :TOUT])
